# revision 9
# baseline (speedup 1.0000x reference)
"""Expert-choice MoE layer on 8 Trainium2 NeuronCores (expert parallelism).

Contract: kernel(**inputs) takes the FULL fp32 inputs of the reference
(x [4,2048,1024], gate_w [1024,8], w1 [8,1024,2048], w2 [8,2048,1024],
w3 [8,1024,2048]) and returns the FULL output, matching
reference.reference(): (y [4,2048,1024] fp32, 0.0).

Per-core plan (core c owns expert e=c):
  1. fp32 gate matmul over this core's 1024-token slice -> logitsT [8,1024]
  2. AllToAll exchanges expert rows -> this core holds logits for its
     expert over ALL 8192 tokens
  3. exact top-1280 threshold via 44-step vectorized bisection
  4. compaction of selected (token_id, score-tau) via gpsimd sparse_gather
  5. token dispatch via gpsimd ap_gather from an SBUF-resident bf16 copy
     of x (h-interleaved plane layout)
  6. grouped SwiGLU FFN in bf16 on the tensor engine, fp32 PSUM
  7. per-expert softmax weights (fp32) folded into the PSUM->SBUF copy
  8. outputs: weighted rows [1280,1024] fp32 + indices; host scatter-adds
"""
import os
import numpy as np
import ml_dtypes

import concourse.bacc as bacc
import concourse.mybir as mybir
from concourse import tile
from concourse import library_config
from concourse.tile import add_dep_helper
from concourse.bass_utils import run_bass_kernel_spmd

B, S, H, E, F = 4, 2048, 1024, 8, 2048
T = B * S                  # 8192
C = 1280                   # capacity = ceil(T/E * 1.25)
TS = T // E                # tokens per core slice
N_ITERS = 30

f32 = mybir.dt.float32
bf16 = mybir.dt.bfloat16
i16 = mybir.dt.int16
u32 = mybir.dt.uint32
u8 = mybir.dt.uint8
Alu = mybir.AluOpType
Act = mybir.ActivationFunctionType

_COMPILED = None
LAST_RESULT = None


def _install_trace_shim():
    """Provide antenv.axon_hooks + NTFF hook so trace=True works under axon."""
    import sys, types
    try:
        import antenv.axon_hooks  # noqa: F401
        return
    except ImportError:
        pass
    try:
        import antenv
        mod = types.ModuleType('antenv.axon_hooks')
        mod._hook = None
        mod.set_axon_ntff_profile_hook = lambda h: setattr(mod, '_hook', h)
        mod.get_axon_ntff_profile_hook = lambda: mod._hook
        sys.modules['antenv.axon_hooks'] = mod
        antenv.axon_hooks = mod
        sys.path.insert(0, '/root/.axon_site/trn_agent_boot')
        import trn_boot
        hook = trn_boot._ntff_profile_via_ctypes('/opt/axon/libaxon_pjrt.so')
        mod.set_axon_ntff_profile_hook(hook)
    except Exception:
        pass


def _build():
    nc = bacc.Bacc("TRN2", target_bir_lowering=False, debug=False, num_devices=8)

    # ---------------- inputs ----------------
    xTs_d = nc.dram_tensor("xTs", [128, 8, TS], f32, kind="ExternalInput")
    gw_d = nc.dram_tensor("gw", [128, 8, E], f32, kind="ExternalInput")
    xp_d = nc.dram_tensor("xp", [4, 128, T, 2], bf16, kind="ExternalInput")
    w1_d = nc.dram_tensor("w1t", [16, 128, 8, 128], bf16, kind="ExternalInput")
    w3_d = nc.dram_tensor("w3t", [16, 128, 8, 128], bf16, kind="ExternalInput")
    w2_d = nc.dram_tensor("w2t", [128, 16, 1024], bf16, kind="ExternalInput")
    iota_d = nc.dram_tensor("iota16", [16, 512], f32, kind="ExternalInput")
    ones_d = nc.dram_tensor("ones128", [128, 128], f32, kind="ExternalInput")
    # ---------------- outputs ----------------
    out_d = nc.dram_tensor("out_tok", [C, H], f32, kind="ExternalOutput")
    idx_d = nc.dram_tensor("idx_out", [16, 80], i16, kind="ExternalOutput")
    nf_d = nc.dram_tensor("nf_out", [1, 2], u32, kind="ExternalOutput")

    with tile.TileContext(nc) as tc:
        with (
            tc.tile_pool(name="const", bufs=1) as cp,
            tc.tile_pool(name="rt", bufs=1) as rt,
            tc.tile_pool(name="dram", bufs=1, space="DRAM") as dp,
            tc.tile_pool(name="psr", bufs=2, space="PSUM") as psr,
        ):
            iota16 = cp.tile([16, 512], f32)
            nc.sync.dma_start(iota16[:], iota_d.ap())
            ones128 = cp.tile([128, 128], f32)
            nc.sync.dma_start(ones128[:], ones_d.ap())

            # ---------------- gate (fp32, this core's slice) ----------------
            with (
                tc.tile_pool(name="gate", bufs=1) as gp,
                tc.tile_pool(name="gatex", bufs=3) as gx,
            ):
                gw = gp.tile([128, 8, E], f32)
                nc.sync.dma_start(gw[:], gw_d.ap())
                lgT = gp.tile([8, TS], f32)
                GCH = 256
                for n in range(TS // GCH):
                    xTsc = gx.tile([128, 8, GCH], f32, name="xTsc")
                    nc.sync.dma_start(xTsc[:], xTs_d[:, :, n * GCH:(n + 1) * GCH])
                    psg = psr.tile([8, 512], f32, name="ps")
                    for k in range(8):
                        nc.tensor.matmul(psg[:, :GCH], gw[:, k, :], xTsc[:, k, :],
                                         start=(k == 0), stop=(k == 7))
                    nc.scalar.activation(lgT[:, n * GCH:(n + 1) * GCH],
                                         psg[:, :GCH], Act.Copy)
                a2a_in = dp.tile([8, TS], f32)
                nc.sync.dma_start(a2a_in[:], lgT[:])
                a2a_out = dp.tile([8, TS], f32)
                nc.gpsimd.collective_compute(
                    "AllToAll", Alu.bypass, replica_groups=[list(range(8))],
                    ins=[a2a_in.opt()], outs=[a2a_out.opt()])

            # x for dispatch: chunked ring, loads start immediately
            xpool_cm = tc.tile_pool(name="xp", bufs=3)
            xpool = xpool_cm.__enter__()
            xchunks = []
            for k in range(4):
                xpc = xpool.tile([128, T, 2], bf16, name="xpc")
                nc.sync.dma_start(xpc[:], xp_d[k])
                xchunks.append(xpc)

            # ---------------- routing ----------------
            lg128 = rt.tile([128, 64], f32)
            nc.sync.dma_start(lg128[:],
                              a2a_out[:].rearrange("a (g f) -> (a g) f", f=64))
            lgw16 = rt.tile([16, 512], f32)
            nc.sync.dma_start(lgw16[:],
                              a2a_out[:].rearrange("a (f2 p) -> p (a f2)", p=16))

            ones_bf = rt.tile([128, 128], bf16)
            nc.vector.tensor_copy(ones_bf[:], ones128[:])
            am = rt.tile([128, 1], f32)
            nc.vector.tensor_reduce(am[:], lg128[:], axis=mybir.AxisListType.X,
                                    op=Alu.max, apply_absolute_value=True)
            ps_s = psr.tile([128, 1], f32, name="ps")
            nc.tensor.matmul(ps_s[:], ones128[:], am[:], start=True, stop=True)
            lo = rt.tile([128, 1], f32)
            hi = rt.tile([128, 1], f32)
            nc.scalar.activation(hi[:], ps_s[:], Act.Copy, bias=1.0, scale=1.0)
            nc.scalar.activation(lo[:], ps_s[:], Act.Copy, bias=-1.0, scale=-1.0)

            tau = rt.tile([128, 1], f32)
            tmp = rt.tile([128, 1], f32)
            mscr = rt.tile([128, 64], f32)
            cntp = rt.tile([128, 1], f32)
            pred = rt.tile([128, 1], u8)
            npred = rt.tile([128, 1], u8)
            cntb = rt.tile([128, 1], bf16)
            for _ in range(N_ITERS):
                nc.vector.tensor_scalar(tau[:], lo[:], hi[:, 0:1], 0.5,
                                        op0=Alu.add, op1=Alu.mult)
                nc.vector.tensor_scalar(mscr[:], lg128[:], tau[:], 0.0,
                                        op0=Alu.is_ge, op1=Alu.add,
                                        accum_out=cntb[:])
                ps_t = psr.tile([128, 1], f32, name="ps")
                nc.tensor.matmul(ps_t[:], ones_bf[:], cntb[:], start=True,
                                 stop=True)
                nc.vector.tensor_scalar(pred[:], ps_t[:], float(C), None,
                                        op0=Alu.is_ge)
                nc.vector.tensor_scalar(npred[:], ps_t[:], float(C), None,
                                        op0=Alu.is_lt)
                nc.vector.copy_predicated(lo[:], pred[:], tau[:])
                nc.vector.copy_predicated(hi[:], npred[:], tau[:])

            msk16 = rt.tile([16, 512], u8)
            nc.vector.tensor_scalar(msk16[:], lgw16[:], lo[:16, :], None,
                                    op0=Alu.is_ge)
            midx = rt.tile([16, 512], f32)
            nc.vector.memset(midx[:], -1.0)
            nc.vector.copy_predicated(midx[:], msk16[:], iota16[:])
            sub16 = rt.tile([16, 512], f32)
            nc.vector.tensor_scalar(sub16[:], lgw16[:], lo[:16, :], None,
                                    op0=Alu.subtract)
            scrm = rt.tile([16, 512], f32)
            nc.vector.memset(scrm[:], -1.0)
            nc.vector.copy_predicated(scrm[:], msk16[:], sub16[:])

            idxf = rt.tile([16, 80], f32)
            scrf = rt.tile([16, 80], f32)
            nf = rt.tile([1, 2], u32)
            ld1 = nc.gpsimd.load_library(library_config.sparse_gather)
            sg1 = nc.gpsimd.sparse_gather(idxf[:], midx[:], num_found=nf[:, 0:1])
            sg2 = nc.gpsimd.sparse_gather(scrf[:], scrm[:], num_found=nf[:, 1:2])
            add_dep_helper(sg1.ins, ld1.ins, reason="lib order")
            add_dep_helper(sg2.ins, ld1.ins, reason="lib order")
            nc.sync.dma_start(nf_d.ap(), nf[:])

            idx16 = rt.tile([16, 80], i16)
            nc.vector.tensor_copy(idx16[:], idxf[:])
            nc.sync.dma_start(idx_d.ap(), idx16[:])
            idx_dram = dp.tile([16, 80], i16)
            nc.sync.dma_start(idx_dram[:], idx16[:])
            idx_rep = rt.tile([128, 80], i16)
            for k in range(8):
                nc.sync.dma_start(idx_rep[16 * k:16 * (k + 1), :], idx_dram[:])

            # ---------------- softmax weights (fp32) ----------------
            expd = rt.tile([16, 80], f32)
            sume = rt.tile([16, 1], f32)
            nc.scalar.activation(expd[:], scrf[:], Act.Exp, accum_out=sume[:])
            ps_tot = psr.tile([1, 1], f32, name="ps")
            nc.tensor.matmul(ps_tot[:], ones128[:16, 0:1], sume[:],
                             start=True, stop=True)
            rtot = rt.tile([1, 1], f32)
            nc.vector.reciprocal(rtot[:], ps_tot[:])
            ps_rb = psr.tile([128, 1], f32, name="ps")
            nc.tensor.matmul(ps_rb[:], ones128[0:1, :], rtot[:],
                             start=True, stop=True)
            rtot128 = rt.tile([128, 1], f32)
            nc.vector.tensor_copy(rtot128[:], ps_rb[:])
            w_flat = dp.tile([1280], f32)
            # write in logical compact order: slot i lives at (i%16, i//16)
            nc.sync.dma_start(w_flat[:].rearrange("(s p) -> p s", p=16), expd[:])
            wnat = rt.tile([128, 10], f32)
            nc.sync.dma_start(wnat[:],
                              w_flat[:].rearrange("(j q) -> q j", q=128))
            wdiv = rt.tile([128, 10], f32)
            nc.vector.tensor_scalar_mul(wdiv[:], wnat[:], rtot128[:])

            # ---------------- dispatch gather (4 h-plane chunks) ----------------
            ld2 = nc.gpsimd.load_library(library_config.ap_gather)
            add_dep_helper(ld2.ins, sg1.ins, reason="lib order")
            add_dep_helper(ld2.ins, sg2.ins, reason="lib order")
            xgrs = []
            prev = ld2
            for k in range(4):
                xgr = rt.tile([128, C, 2], bf16, name=f"xgr{k}")
                g = nc.gpsimd.ap_gather(xgr[:], xchunks[k][:], idx_rep[:],
                                        channels=128, num_elems=T, d=2,
                                        num_idxs=C)
                add_dep_helper(g.ins, prev.ins, reason="lib order")
                prev = g
                xgrs.append(xgr)

            xpool_cm.__exit__(None, None, None)  # release x ring

            # ---------------- FFN ----------------
            CCH = [(0, 512), (512, 512), (1024, 256)]
            with (
                tc.tile_pool(name="wring", bufs=3) as wr,
                tc.tile_pool(name="hbuf", bufs=1) as hb,
                tc.tile_pool(name="oring", bufs=3) as orp,
                tc.tile_pool(name="psff", bufs=2, space="PSUM") as psf,
                tc.tile_pool(name="psff2", bufs=1, space="PSUM") as psf2,
            ):
                w2sb = hb.tile([128, 16, 1024], bf16)
                nc.sync.dma_start(w2sb[:], w2_d.ap())
                xg = hb.tile([128, 8, C], bf16)
                for k in range(4):
                    for c in range(2):
                        nc.vector.tensor_copy(xg[:, 2 * k + c, :],
                                              xgrs[k][:, :, c])
                hT = hb.tile([128, 16, C], bf16)
                for f in range(16):
                    w1sb = wr.tile([128, 8, 128], bf16, name="w1sb")
                    nc.sync.dma_start(w1sb[:], w1_d[f])
                    w3sb = wr.tile([128, 8, 128], bf16, name="w3sb")
                    nc.sync.dma_start(w3sb[:], w3_d[f])
                    for (c0, cw) in CCH:
                        psa = psf.tile([128, 512], f32, name="psa")
                        for c8 in range(8):
                            nc.tensor.matmul(psa[:, :cw], w1sb[:, c8, :],
                                             xg[:, c8, c0:c0 + cw],
                                             start=(c8 == 0), stop=(c8 == 7))
                        psg2 = psf.tile([128, 512], f32, name="psg2")
                        for c8 in range(8):
                            nc.tensor.matmul(psg2[:, :cw], w3sb[:, c8, :],
                                             xg[:, c8, c0:c0 + cw],
                                             start=(c8 == 0), stop=(c8 == 7))
                        sil = orp.tile([128, 512], f32, name="sil")
                        nc.scalar.activation(sil[:, :cw], psa[:, :cw], Act.Silu)
                        nc.vector.tensor_mul(hT[:, f, c0:c0 + cw], sil[:, :cw],
                                             psg2[:, :cw])

                for j in range(10):
                    osb = orp.tile([128, 1024], f32, name="osb")
                    pso0 = psf2.tile([128, 512], f32, name="pso0")
                    pso1 = psf2.tile([128, 512], f32, name="pso1")
                    for k2 in range(16):
                        lhs = hT[:, k2, j * 128:(j + 1) * 128]
                        nc.tensor.matmul(pso0[:], lhs, w2sb[:, k2, 0:512],
                                         start=(k2 == 0), stop=(k2 == 15))
                        nc.tensor.matmul(pso1[:], lhs, w2sb[:, k2, 512:1024],
                                         start=(k2 == 0), stop=(k2 == 15))
                    nc.scalar.activation(osb[:, 0:512], pso0[:], Act.Copy,
                                         scale=wdiv[:, j:j + 1])
                    nc.scalar.activation(osb[:, 512:1024], pso1[:], Act.Copy,
                                         scale=wdiv[:, j:j + 1])
                    nc.sync.dma_start(
                        out_d.rearrange("(j p) h -> p j h", p=128)[:, j, :],
                        osb[:])
    nc.compile()
    return nc


def _prep_inputs(x, gate_w, w1, w2, w3):
    x2 = np.ascontiguousarray(np.asarray(x, np.float32).reshape(T, H))
    gate_w = np.asarray(gate_w, np.float32)
    xp = np.ascontiguousarray(
        x2.astype(ml_dtypes.bfloat16).reshape(T, 128, 4, 2)
        .transpose(2, 0, 1, 3).transpose(0, 2, 1, 3))
    gw_t = np.ascontiguousarray(gate_w.reshape(8, 128, E).transpose(1, 0, 2))
    iota16 = np.ascontiguousarray(
        np.arange(T, dtype=np.float32).reshape(512, 16).T)
    ones128 = np.ones((128, 128), np.float32)
    in_maps = []
    for c in range(E):
        xTs = np.ascontiguousarray(
            x2[c * TS:(c + 1) * TS].T.reshape(8, 128, TS).transpose(1, 0, 2))
        w1f = np.ascontiguousarray(
            np.asarray(w1[c], np.float32).astype(ml_dtypes.bfloat16)
            .reshape(128, 8, 16, 128).transpose(2, 0, 1, 3))
        w3f = np.ascontiguousarray(
            np.asarray(w3[c], np.float32).astype(ml_dtypes.bfloat16)
            .reshape(128, 8, 16, 128).transpose(2, 0, 1, 3))
        w2t = np.ascontiguousarray(
            np.asarray(w2[c], np.float32).astype(ml_dtypes.bfloat16)
            .reshape(16, 128, 1024).transpose(1, 0, 2))
        in_maps.append({
            "xTs": xTs, "gw": gw_t, "xp": xp, "w1t": w1f, "w3t": w3f,
            "w2t": w2t, "iota16": iota16, "ones128": ones128,
        })
    return in_maps


def kernel(x, gate_w, w1, w2, w3):
    global _COMPILED, LAST_RESULT
    trace = bool(os.environ.get("KERNEL_TRACE"))
    if trace:
        _install_trace_shim()
    if _COMPILED is None:
        _COMPILED = _build()
    in_maps = _prep_inputs(x, gate_w, w1, w2, w3)
    res = run_bass_kernel_spmd(_COMPILED, in_maps, list(range(E)), trace=trace)
    LAST_RESULT = res
    y = np.zeros((T, H), np.float32)
    for c in range(E):
        r = res.results[c]
        nf = r["nf_out"]
        assert nf[0, 0] == C and nf[0, 1] == C, f"core {c} num_found {nf}"
        idx = r["idx_out"].T.reshape(-1).astype(np.int64)
        y[idx] += r["out_tok"]
    return y.reshape(B, S, H), np.float32(0.0)


# revision 17
# speedup vs baseline: 1.2036x; 1.2036x over previous
"""Expert-choice MoE layer on 8 Trainium2 NeuronCores (expert parallelism).

Contract: kernel(**inputs) takes the FULL fp32 inputs of the reference
(x [4,2048,1024], gate_w [1024,8], w1 [8,1024,2048], w2 [8,2048,1024],
w3 [8,1024,2048]) and returns the FULL output, matching
reference.reference(): (y [4,2048,1024] fp32, 0.0).

Per-core plan (core c owns expert e=c):
  1. fp32 gate matmul over this core's 1024-token slice -> logitsT [8,1024]
  2. AllToAll exchanges expert rows -> this core holds logits for its
     expert over ALL 8192 tokens
  3. exact top-1280 threshold via 44-step vectorized bisection
  4. compaction of selected (token_id, score-tau) via gpsimd sparse_gather
  5. token dispatch via gpsimd ap_gather from an SBUF-resident bf16 copy
     of x (h-interleaved plane layout)
  6. grouped SwiGLU FFN in bf16 on the tensor engine, fp32 PSUM
  7. per-expert softmax weights (fp32) folded into the PSUM->SBUF copy
  8. outputs: weighted rows [1280,1024] fp32 + indices; host scatter-adds
"""
import os
import numpy as np
import ml_dtypes

import concourse.bacc as bacc
import concourse.mybir as mybir
from concourse import tile
from concourse import library_config
from concourse.tile import add_dep_helper
from concourse.bass_utils import run_bass_kernel_spmd

B, S, H, E, F = 4, 2048, 1024, 8, 2048
T = B * S                  # 8192
C = 1280                   # capacity = ceil(T/E * 1.25)
TS = T // E                # tokens per core slice
N_ITERS = 28

f32 = mybir.dt.float32
bf16 = mybir.dt.bfloat16
i16 = mybir.dt.int16
u32 = mybir.dt.uint32
u8 = mybir.dt.uint8
Alu = mybir.AluOpType
Act = mybir.ActivationFunctionType

_COMPILED = None
LAST_RESULT = None


def _install_trace_shim():
    """Provide antenv.axon_hooks + NTFF hook so trace=True works under axon."""
    import sys, types
    try:
        import antenv.axon_hooks  # noqa: F401
        return
    except ImportError:
        pass
    try:
        import antenv
        mod = types.ModuleType('antenv.axon_hooks')
        mod._hook = None
        mod.set_axon_ntff_profile_hook = lambda h: setattr(mod, '_hook', h)
        mod.get_axon_ntff_profile_hook = lambda: mod._hook
        sys.modules['antenv.axon_hooks'] = mod
        antenv.axon_hooks = mod
        sys.path.insert(0, '/root/.axon_site/trn_agent_boot')
        import trn_boot
        hook = trn_boot._ntff_profile_via_ctypes('/opt/axon/libaxon_pjrt.so')
        mod.set_axon_ntff_profile_hook(hook)
    except Exception:
        pass


def _build():
    nc = bacc.Bacc("TRN2", target_bir_lowering=False, debug=False, num_devices=8)

    # ---------------- inputs ----------------
    xTs_d = nc.dram_tensor("xTs", [128, 8, TS], f32, kind="ExternalInput")
    gw_d = nc.dram_tensor("gw", [128, 8, E], f32, kind="ExternalInput")
    xp_d = nc.dram_tensor("xp", [128, T, 8], bf16, kind="ExternalInput")
    w1_d = nc.dram_tensor("w1t", [16, 128, 8, 128], bf16, kind="ExternalInput")
    w3_d = nc.dram_tensor("w3t", [16, 128, 8, 128], bf16, kind="ExternalInput")
    w2_d = nc.dram_tensor("w2t", [128, 16, 1024], bf16, kind="ExternalInput")
    iota_d = nc.dram_tensor("iota16", [16, 512], f32, kind="ExternalInput")
    ones_d = nc.dram_tensor("ones128", [128, 128], f32, kind="ExternalInput")
    # ---------------- outputs ----------------
    out_d = nc.dram_tensor("out_tok", [C, H], f32, kind="ExternalOutput")
    idx_d = nc.dram_tensor("idx_out", [16, 80], i16, kind="ExternalOutput")
    nf_d = nc.dram_tensor("nf_out", [1, 2], u32, kind="ExternalOutput")

    with tile.TileContext(nc) as tc:
        with (
            tc.tile_pool(name="const", bufs=1) as cp,
            tc.tile_pool(name="rt", bufs=1) as rt,
            tc.tile_pool(name="dram", bufs=1, space="DRAM") as dp,
            tc.tile_pool(name="psr", bufs=2, space="PSUM") as psr,
        ):
            iota16 = cp.tile([16, 512], f32)
            nc.sync.dma_start(iota16[:], iota_d.ap())
            ones128 = cp.tile([128, 128], f32)
            nc.sync.dma_start(ones128[:], ones_d.ap())

            # ---------------- gate (fp32, this core's slice) ----------------
            with (
                tc.tile_pool(name="gate", bufs=1) as gp,
                tc.tile_pool(name="gatex", bufs=3) as gx,
            ):
                gw = gp.tile([128, 8, E], f32)
                nc.sync.dma_start(gw[:], gw_d.ap())
                lgT = gp.tile([8, TS], f32)
                GCH = 256
                for n in range(TS // GCH):
                    xTsc = gx.tile([128, 8, GCH], f32, name="xTsc")
                    nc.sync.dma_start(xTsc[:], xTs_d[:, :, n * GCH:(n + 1) * GCH])
                    psg = psr.tile([8, 512], f32, name="ps")
                    for k in range(8):
                        nc.tensor.matmul(psg[:, :GCH], gw[:, k, :], xTsc[:, k, :],
                                         start=(k == 0), stop=(k == 7))
                    nc.scalar.activation(lgT[:, n * GCH:(n + 1) * GCH],
                                         psg[:, :GCH], Act.Copy)
                a2a_in = dp.tile([8, TS], f32)
                nc.sync.dma_start(a2a_in[:], lgT[:])
                a2a_out = dp.tile([8, TS], f32)
                nc.gpsimd.collective_compute(
                    "AllToAll", Alu.bypass, replica_groups=[list(range(8))],
                    ins=[a2a_in.opt()], outs=[a2a_out.opt()])

            # x for dispatch: big load, consumed by the gathers
            xpool_cm = tc.tile_pool(name="xp", bufs=1)
            xpool = xpool_cm.__enter__()
            xp = xpool.tile([128, T, 8], bf16)
            nc.sync.dma_start(xp[:], xp_d.ap())

            # ---------------- routing ----------------
            lg128 = rt.tile([128, 64], f32)
            nc.sync.dma_start(lg128[:],
                              a2a_out[:].rearrange("a (g f) -> (a g) f", f=64))
            lgw16 = rt.tile([16, 512], f32)
            nc.sync.dma_start(lgw16[:],
                              a2a_out[:].rearrange("a (f2 p) -> p (a f2)", p=16))

            ones_bf = rt.tile([128, 128], bf16)
            nc.vector.tensor_copy(ones_bf[:], ones128[:])
            am = rt.tile([128, 1], f32)
            nc.vector.tensor_reduce(am[:], lg128[:], axis=mybir.AxisListType.X,
                                    op=Alu.max, apply_absolute_value=True)
            ps_s = psr.tile([128, 1], f32, name="ps")
            nc.tensor.matmul(ps_s[:], ones128[:], am[:], start=True, stop=True)
            lo = rt.tile([128, 1], f32)
            hi = rt.tile([128, 1], f32)
            nc.scalar.activation(hi[:], ps_s[:], Act.Copy, bias=1.0, scale=1.0)
            nc.scalar.activation(lo[:], ps_s[:], Act.Copy, bias=-1.0, scale=-1.0)

            tau = rt.tile([128, 1], f32)
            tmp = rt.tile([128, 1], f32)
            mscr = rt.tile([128, 64], f32)
            cntp = rt.tile([128, 1], f32)
            pred = rt.tile([128, 1], u8)
            npred = rt.tile([128, 1], u8)
            cntb = rt.tile([128, 1], bf16)
            for _ in range(N_ITERS):
                nc.vector.tensor_scalar(tau[:], lo[:], hi[:, 0:1], 0.5,
                                        op0=Alu.add, op1=Alu.mult)
                nc.vector.tensor_scalar(mscr[:], lg128[:], tau[:], 0.0,
                                        op0=Alu.is_ge, op1=Alu.add,
                                        accum_out=cntb[:])
                ps_t = psr.tile([128, 1], f32, name="ps")
                nc.tensor.matmul(ps_t[:], ones_bf[:], cntb[:], start=True,
                                 stop=True)
                nc.vector.tensor_scalar(pred[:], ps_t[:], float(C), None,
                                        op0=Alu.is_ge)
                nc.vector.tensor_scalar(npred[:], ps_t[:], float(C), None,
                                        op0=Alu.is_lt)
                nc.vector.copy_predicated(lo[:], pred[:], tau[:])
                nc.vector.copy_predicated(hi[:], npred[:], tau[:])

            msk16 = rt.tile([16, 512], u8)
            nc.vector.tensor_scalar(msk16[:], lgw16[:], lo[:16, :], None,
                                    op0=Alu.is_ge)
            midx = rt.tile([16, 512], f32)
            nc.vector.memset(midx[:], -1.0)
            nc.vector.copy_predicated(midx[:], msk16[:], iota16[:])
            sub16 = rt.tile([16, 512], f32)
            nc.vector.tensor_scalar(sub16[:], lgw16[:], lo[:16, :], None,
                                    op0=Alu.subtract)
            scrm = rt.tile([16, 512], f32)
            nc.vector.memset(scrm[:], -1.0)
            nc.vector.copy_predicated(scrm[:], msk16[:], sub16[:])

            idxf = rt.tile([16, 80], f32)
            scrf = rt.tile([16, 80], f32)
            nf = rt.tile([1, 2], u32)
            ld1 = nc.gpsimd.load_library(library_config.sparse_gather)
            sg1 = nc.gpsimd.sparse_gather(idxf[:], midx[:], num_found=nf[:, 0:1])
            sg2 = nc.gpsimd.sparse_gather(scrf[:], scrm[:], num_found=nf[:, 1:2])
            add_dep_helper(sg1.ins, ld1.ins, sync=False, reason="lib order")
            add_dep_helper(sg2.ins, ld1.ins, sync=False, reason="lib order")
            nc.sync.dma_start(nf_d.ap(), nf[:])

            idx16 = rt.tile([16, 80], i16)
            nc.vector.tensor_copy(idx16[:], idxf[:])
            nc.sync.dma_start(idx_d.ap(), idx16[:])
            idx_dram = dp.tile([16, 80], i16)
            nc.sync.dma_start(idx_dram[:], idx16[:])
            idx_rep = rt.tile([128, 80], i16)
            for k in range(8):
                nc.sync.dma_start(idx_rep[16 * k:16 * (k + 1), :], idx_dram[:])

            # ---------------- softmax weights (fp32) ----------------
            expd = rt.tile([16, 80], f32)
            sume = rt.tile([16, 1], f32)
            nc.scalar.activation(expd[:], scrf[:], Act.Exp, accum_out=sume[:])
            ps_tot = psr.tile([1, 1], f32, name="ps")
            nc.tensor.matmul(ps_tot[:], ones128[:16, 0:1], sume[:],
                             start=True, stop=True)
            rtot = rt.tile([1, 1], f32)
            nc.vector.reciprocal(rtot[:], ps_tot[:])
            ps_rb = psr.tile([128, 1], f32, name="ps")
            nc.tensor.matmul(ps_rb[:], ones128[0:1, :], rtot[:],
                             start=True, stop=True)
            rtot128 = rt.tile([128, 1], f32)
            nc.vector.tensor_copy(rtot128[:], ps_rb[:])
            w_flat = dp.tile([1280], f32)
            # write in logical compact order: slot i lives at (i%16, i//16)
            nc.sync.dma_start(w_flat[:].rearrange("(s p) -> p s", p=16), expd[:])
            wnat = rt.tile([128, 10], f32)
            nc.sync.dma_start(wnat[:],
                              w_flat[:].rearrange("(j q) -> q j", q=128))
            wdiv = rt.tile([128, 10], f32)
            nc.vector.tensor_scalar_mul(wdiv[:], wnat[:], rtot128[:])

            # ------- dispatch gathers: one per 512-token compact chunk -------
            ld2 = nc.gpsimd.load_library(library_config.ap_gather)
            add_dep_helper(ld2.ins, sg1.ins, reason="lib order")
            add_dep_helper(ld2.ins, sg2.ins, reason="lib order")
            xgr_full = rt.tile([128, C, 8], bf16)
            g = nc.gpsimd.ap_gather(xgr_full[:], xp[:], idx_rep[:],
                                    channels=128, num_elems=T, d=8,
                                    num_idxs=C)
            add_dep_helper(g.ins, ld2.ins, reason="lib order")
            CCH = [(0, 512), (512, 512), (1024, 256)]
            xgrs = [xgr_full[:, c0:c0 + cw, :] for (c0, cw) in CCH]

            xpool_cm.__exit__(None, None, None)  # release 16MB x buffer

            # ---------------- FFN ----------------
            CCH = [(0, 512), (512, 512), (1024, 256)]
            with (
                tc.tile_pool(name="wring", bufs=3) as wr,
                tc.tile_pool(name="hbuf", bufs=1) as hb,
                tc.tile_pool(name="oring", bufs=3) as orp,
                tc.tile_pool(name="psff", bufs=2, space="PSUM") as psf,
                tc.tile_pool(name="psff2", bufs=1, space="PSUM") as psf2,
            ):
                w2sb = hb.tile([128, 16, 1024], bf16)
                nc.sync.dma_start(w2sb[:], w2_d.ap())
                xg = hb.tile([128, 8, C], bf16)
                for ci, (c0, cw) in enumerate(CCH):
                    for c8 in range(8):
                        nc.vector.tensor_copy(xg[:, c8, c0:c0 + cw],
                                              xgrs[ci][:, :cw, c8])
                hT = hb.tile([128, 16, C], bf16)
                for ci, (c0, cw) in enumerate(CCH):
                    for f in range(16):
                        w1sb = wr.tile([128, 8, 128], bf16, name="w1sb")
                        nc.sync.dma_start(w1sb[:], w1_d[f])
                        w3sb = wr.tile([128, 8, 128], bf16, name="w3sb")
                        nc.sync.dma_start(w3sb[:], w3_d[f])
                        psa = psf.tile([128, 512], f32, name="psa")
                        for c8 in range(8):
                            nc.tensor.matmul(psa[:, :cw], w1sb[:, c8, :],
                                             xg[:, c8, c0:c0 + cw],
                                             start=(c8 == 0), stop=(c8 == 7))
                        psg2 = psf.tile([128, 512], f32, name="psg2")
                        for c8 in range(8):
                            nc.tensor.matmul(psg2[:, :cw], w3sb[:, c8, :],
                                             xg[:, c8, c0:c0 + cw],
                                             start=(c8 == 0), stop=(c8 == 7))
                        sil = orp.tile([128, 512], f32, name="sil")
                        nc.scalar.activation(sil[:, :cw], psa[:, :cw], Act.Silu)
                        nc.vector.tensor_mul(hT[:, f, c0:c0 + cw], sil[:, :cw],
                                             psg2[:, :cw])

                for j in range(10):
                    osb = orp.tile([128, 1024], f32, name="osb")
                    pso0 = psf2.tile([128, 512], f32, name="pso0")
                    pso1 = psf2.tile([128, 512], f32, name="pso1")
                    for k2 in range(16):
                        lhs = hT[:, k2, j * 128:(j + 1) * 128]
                        nc.tensor.matmul(pso0[:], lhs, w2sb[:, k2, 0:512],
                                         start=(k2 == 0), stop=(k2 == 15))
                        nc.tensor.matmul(pso1[:], lhs, w2sb[:, k2, 512:1024],
                                         start=(k2 == 0), stop=(k2 == 15))
                    nc.scalar.activation(osb[:, 0:512], pso0[:], Act.Copy,
                                         scale=wdiv[:, j:j + 1])
                    nc.scalar.activation(osb[:, 512:1024], pso1[:], Act.Copy,
                                         scale=wdiv[:, j:j + 1])
                    nc.sync.dma_start(
                        out_d.rearrange("(j p) h -> p j h", p=128)[:, j, :],
                        osb[:])
    nc.compile()
    return nc


def _prep_inputs(x, gate_w, w1, w2, w3):
    x2 = np.ascontiguousarray(np.asarray(x, np.float32).reshape(T, H))
    gate_w = np.asarray(gate_w, np.float32)
    xp = np.ascontiguousarray(
        x2.astype(ml_dtypes.bfloat16).reshape(T, 128, 8).transpose(1, 0, 2))
    gw_t = np.ascontiguousarray(gate_w.reshape(8, 128, E).transpose(1, 0, 2))
    iota16 = np.ascontiguousarray(
        np.arange(T, dtype=np.float32).reshape(512, 16).T)
    ones128 = np.ones((128, 128), np.float32)
    in_maps = []
    for c in range(E):
        xTs = np.ascontiguousarray(
            x2[c * TS:(c + 1) * TS].T.reshape(8, 128, TS).transpose(1, 0, 2))
        w1f = np.ascontiguousarray(
            np.asarray(w1[c], np.float32).astype(ml_dtypes.bfloat16)
            .reshape(128, 8, 16, 128).transpose(2, 0, 1, 3))
        w3f = np.ascontiguousarray(
            np.asarray(w3[c], np.float32).astype(ml_dtypes.bfloat16)
            .reshape(128, 8, 16, 128).transpose(2, 0, 1, 3))
        w2t = np.ascontiguousarray(
            np.asarray(w2[c], np.float32).astype(ml_dtypes.bfloat16)
            .reshape(16, 128, 1024).transpose(1, 0, 2))
        in_maps.append({
            "xTs": xTs, "gw": gw_t, "xp": xp, "w1t": w1f, "w3t": w3f,
            "w2t": w2t, "iota16": iota16, "ones128": ones128,
        })
    return in_maps


def kernel(x, gate_w, w1, w2, w3):
    global _COMPILED, LAST_RESULT
    trace = bool(os.environ.get("KERNEL_TRACE"))
    if trace:
        _install_trace_shim()
    if _COMPILED is None:
        _COMPILED = _build()
    in_maps = _prep_inputs(x, gate_w, w1, w2, w3)
    res = run_bass_kernel_spmd(_COMPILED, in_maps, list(range(E)), trace=trace)
    LAST_RESULT = res
    y = np.zeros((T, H), np.float32)
    for c in range(E):
        r = res.results[c]
        nf = r["nf_out"]
        assert nf[0, 0] == C and nf[0, 1] == C, f"core {c} num_found {nf}"
        idx = r["idx_out"].T.reshape(-1).astype(np.int64)
        y[idx] += r["out_tok"]
    return y.reshape(B, S, H), np.float32(0.0)
